# revision 1
# baseline (speedup 1.0000x reference)
"""Cross-attention (global, batch-flattened K/V) Trainium2 kernel.

Problem: emb [16, 4096, 64]; two cross-attention halves:
  out_l2u = cross(q=emb[:8],  kv=emb[8:])   -> rows 0..7
  out_u2l = cross(q=emb[8:],  kv=emb[:8])   -> rows 8..15
cross(): q/k/v proj (64->512), s = einsum('bnc,nd->bcd', q, kflat),
InstanceNorm over (CH, B*CH) plane per b, softmax over d, ctx = a @ vflat^T,
out = ctx @ Wout.

Sharding: 16 independent (cross, q-batch) instances, 2 per core.
Cores 0-3: q from lower half (kv = upper), cores 4-7: q from upper
(kv = lower), so each core needs k/v projections of one half only.
No collectives; weights replicated.

Per-core dataflow (all matmuls in float32r, 1 cycle/row on the PE):
  phase A: vT[b'] = (kv[b'] @ Wv)^T via PE -> DRAM scratch [8, 512, 4096]
  per instance:
    q = emb_q @ Wq resident in SBUF (lhsT layout via PE transposes)
    s[c, d] accumulated in PSUM over n; k-chunks projected on the fly
      (kf never touches DRAM); stats (sum, sumsq) fused on PSUM drain
    InstanceNorm + exp fused into one ACT pass (scale/bias per partition),
      row-sums via accum_out; softmax division deferred to ctx output
    aT via PE transposes
    ctxT[c, n] accumulated in PSUM over d, vT streamed from DRAM
    out = ctxT^T @ Wout via PE, DMA to output
"""

import numpy as np
import concourse.bass as bass
import concourse.mybir as mybir
import concourse.tile as tile
from concourse import bacc
from concourse.bass_utils import run_bass_kernel_spmd

dt = mybir.dt
AF = mybir.ActivationFunctionType
ALU = mybir.AluOpType

B = 8            # batches per half
N = 4096         # sequence length
C = 64           # embedding channels
CH = 512         # num_heads * C
NB = N // 128    # 32 n-blocks
NCH = N // 512   # 8 chunks of 512
CB = CH // 128   # 4 c-blocks
D = B * CH       # 4096 flattened kv dim
EPS = 1e-5
MM = dt.float32r  # matmul operand dtype
PLANE = float(CH * D)  # InstanceNorm plane size per instance

_nc = None


def _build():
    nc = bacc.Bacc("TRN2", target_bir_lowering=False, debug=False, num_devices=8)

    embq = nc.declare_dram_parameter("embq", [2, N, C], dt.float32, isOutput=False)
    embkv = nc.declare_dram_parameter("embkv", [B, N, C], dt.float32, isOutput=False)
    Wq_d = nc.declare_dram_parameter("Wq", [C, CH], dt.float32, isOutput=False)
    Wk_d = nc.declare_dram_parameter("Wk", [C, CH], dt.float32, isOutput=False)
    Wv_d = nc.declare_dram_parameter("Wv", [C, CH], dt.float32, isOutput=False)
    Wout_d = nc.declare_dram_parameter("Wout", [CH, C], dt.float32, isOutput=False)
    ident_d = nc.declare_dram_parameter("ident", [128, 128], dt.float32, isOutput=False)
    ones_d = nc.declare_dram_parameter("ones", [128, 128], dt.float32, isOutput=False)
    out_d = nc.declare_dram_parameter("out", [2, C, N], dt.float32, isOutput=True)

    vT_dram = nc.dram_tensor("vT_scratch", [B, CH, N], MM)

    with tile.TileContext(nc) as tc:
        with (
            tc.tile_pool(name="const", bufs=1) as constp,
            tc.tile_pool(name="io", bufs=2) as iop,
            tc.tile_pool(name="embt", bufs=1) as embtp,
            tc.tile_pool(name="stream", bufs=4) as streamp,
            tc.tile_pool(name="big", bufs=2) as bigp,
            tc.tile_pool(name="small", bufs=2) as smallp,
            tc.tile_pool(name="ps", bufs=8, space="PSUM") as psp,
        ):
            # ---- constants ----
            ident = constp.tile([128, 128], dt.float32, tag="ident")
            nc.sync.dma_start(ident[:], ident_d[:])
            ones_f = iop.tile([128, 128], dt.float32, tag="wst")
            nc.sync.dma_start(ones_f[:], ones_d[:])
            ones_r = constp.tile([128, 128], MM, tag="ones_r")
            nc.vector.tensor_copy(out=ones_r[:], in_=ones_f[:])

            w_rs = {}
            for name, wd in (("Wq", Wq_d), ("Wk", Wk_d), ("Wv", Wv_d)):
                wst = iop.tile([C, CH], dt.float32, tag="wst")
                nc.sync.dma_start(wst[:], wd[:])
                wr = constp.tile([C, CH], MM, tag=f"{name}_r")
                nc.vector.tensor_copy(out=wr[:], in_=wst[:])
                w_rs[name] = wr
            Wq_r, Wk_r, Wv_r = w_rs["Wq"], w_rs["Wk"], w_rs["Wv"]

            wost = iop.tile([128, CB, C], dt.float32, tag="wst")
            nc.sync.dma_start(
                wost[:], Wout_d[:].rearrange("(cb p) c -> p cb c", p=128)
            )
            Wout_r = constp.tile([128, CB, C], MM, tag="Wout_r")
            nc.vector.tensor_copy(out=Wout_r[:], in_=wost[:])

            # ---- helper: build embT [64, N] (f32r) for one batch ----
            def build_embT(src):  # src: DRAM AP [N, C] fp32
                et = embtp.tile([C, N], MM, tag="embT")
                for h in range(2):
                    lt = iop.tile([128, NB // 2, C], dt.float32, tag="embload")
                    nc.sync.dma_start(
                        lt[:],
                        src[h * (N // 2):(h + 1) * (N // 2), :].rearrange(
                            "(nb p) c -> p nb c", p=128
                        ),
                    )
                    for g in range(4):  # 4 transpose groups of 4 n-blocks
                        pt = psp.tile([128, 512], dt.float32, tag="pp")
                        for j in range(4):
                            nc.tensor.transpose(
                                pt[0:C, j * 128:(j + 1) * 128],
                                lt[:, g * 4 + j, :],
                                ident[:],
                            )
                        base = (h * 16 + g * 4) * 128
                        nc.vector.tensor_copy(
                            out=et[:, base:base + 512], in_=pt[0:C, :]
                        )
                return et

            # ---- phase A: vT for all kv batches -> DRAM ----
            for b in range(B):
                et = build_embT(embkv[b])
                for cb in range(CB):
                    for g in range(NCH):
                        pt = psp.tile([128, 512], dt.float32, tag="pp")
                        nc.tensor.matmul(
                            pt[:],
                            Wv_r[:, cb * 128:(cb + 1) * 128],
                            et[:, g * 512:(g + 1) * 512],
                            start=True,
                            stop=True,
                        )
                        st = streamp.tile([128, 512], MM, tag="vst", bufs=2)
                        nc.vector.tensor_copy(out=st[:], in_=pt[:])
                        nc.sync.dma_start(
                            vT_dram[b, cb * 128:(cb + 1) * 128,
                                    g * 512:(g + 1) * 512],
                            st[:],
                        )

            # ---- per instance ----
            for inst in range(2):
                # q resident: [128, nb, ch] f32r
                et_q = build_embT(embq[inst])
                q_sb = bigp.tile([128, NB, CH], MM, tag="big")
                for nb in range(NB):
                    pt = psp.tile([128, 512], dt.float32, tag="pp")
                    nc.tensor.matmul(
                        pt[:],
                        et_q[:, nb * 128:(nb + 1) * 128],
                        Wq_r[:],
                        start=True,
                        stop=True,
                    )
                    nc.vector.tensor_copy(out=q_sb[:, nb, :], in_=pt[:])

                # s = q^T @ kflat, accumulated over n; k projected on the fly
                s_sb = bigp.tile([128, CB, N], dt.float32, tag="big")
                ssum = smallp.tile([128, CB, B], dt.float32, tag="ssum")
                ssq = smallp.tile([128, CB, B], dt.float32, tag="ssq")
                for db in range(B):
                    et = build_embT(embkv[db])
                    ps_s = [psp.tile([128, 512], dt.float32, tag="pp",
                                     name=f"ps_s{cb_}")
                            for cb_ in range(CB)]
                    for nb in range(NB):
                        ptk = psp.tile([128, 512], dt.float32, tag="pp")
                        nc.tensor.matmul(
                            ptk[:],
                            et[:, nb * 128:(nb + 1) * 128],
                            Wk_r[:],
                            start=True,
                            stop=True,
                        )
                        kf = streamp.tile([128, 512], MM, tag="kf")
                        nc.vector.tensor_copy(out=kf[:], in_=ptk[:])
                        for cb in range(CB):
                            nc.tensor.matmul(
                                ps_s[cb][:],
                                q_sb[:, nb, cb * 128:(cb + 1) * 128],
                                kf[:],
                                start=(nb == 0),
                                stop=(nb == NB - 1),
                            )
                    for cb in range(CB):
                        nc.scalar.activation(
                            s_sb[:, cb, db * 512:(db + 1) * 512],
                            ps_s[cb][:],
                            AF.Copy,
                            accum_out=ssum[:, cb, db:db + 1],
                        )
                        # Square in place on PSUM (after the copy has read it)
                        nc.scalar.activation(
                            ps_s[cb][:],
                            ps_s[cb][:],
                            AF.Square,
                            accum_out=ssq[:, cb, db:db + 1],
                        )

                # ---- InstanceNorm stats -> per-partition scale/bias ----
                red = smallp.tile([128, 2], dt.float32, tag="red")
                nc.vector.tensor_reduce(
                    out=red[:, 0:1], in_=ssum[:], axis=mybir.AxisListType.XY,
                    op=ALU.add,
                )
                nc.vector.tensor_reduce(
                    out=red[:, 1:2], in_=ssq[:], axis=mybir.AxisListType.XY,
                    op=ALU.add,
                )
                red_r = smallp.tile([128, 2], MM, tag="red_r")
                nc.vector.tensor_copy(out=red_r[:], in_=red[:])
                ptr = psp.tile([128, 512], dt.float32, tag="pp")
                # all-partition totals via ones matmul
                nc.tensor.matmul(
                    ptr[:, 0:2], ones_r[:], red_r[:], start=True, stop=True
                )
                stats = smallp.tile([128, 8], dt.float32, tag="stats")
                # mu = tot_sum / PLANE ; ex2 = tot_sq / PLANE
                nc.scalar.activation(
                    stats[:, 0:2], ptr[:, 0:2], AF.Copy, bias=0.0,
                    scale=1.0 / PLANE,
                )
                mu = stats[:, 0:1]
                ex2 = stats[:, 1:2]
                musq = stats[:, 2:3]
                var = stats[:, 3:4]
                std = stats[:, 4:5]
                rstd = stats[:, 5:6]
                nmr = stats[:, 6:7]
                nc.vector.tensor_tensor(out=musq, in0=mu, in1=mu, op=ALU.mult)
                nc.vector.tensor_tensor(out=var, in0=ex2, in1=musq,
                                        op=ALU.subtract)
                nc.vector.tensor_scalar_add(var, var, EPS)
                nc.scalar.activation(std, var, AF.Sqrt, bias=0.0)
                nc.vector.reciprocal(rstd, std)
                nc.vector.tensor_tensor(out=nmr, in0=mu, in1=rstd, op=ALU.mult)
                nc.scalar.mul(nmr, nmr, -1.0)

                # ---- softmax numerator: a = exp((s - mu) * rstd), in place ----
                den = smallp.tile([128, CB], dt.float32, tag="den")
                for cb in range(CB):
                    nc.scalar.activation(
                        s_sb[:, cb, :],
                        s_sb[:, cb, :],
                        AF.Exp,
                        bias=nmr,
                        scale=rstd,
                        accum_out=den[:, cb:cb + 1],
                    )
                inv_den = smallp.tile([128, CB], dt.float32, tag="invden")
                nc.vector.reciprocal(inv_den[:], den[:])

                # ---- aT via PE transposes ----
                aT = bigp.tile([128, NB, CH], MM, tag="big")
                for ds in range(NB):
                    for cb in range(CB):
                        pt = psp.tile([128, 512], dt.float32, tag="pp")
                        nc.tensor.transpose(
                            pt[:, 0:128],
                            s_sb[:, cb, ds * 128:(ds + 1) * 128],
                            ident[:],
                        )
                        nc.vector.tensor_copy(
                            out=aT[:, ds, cb * 128:(cb + 1) * 128],
                            in_=pt[:, 0:128],
                        )

                # ---- ctxT = (a @ vflat^T) / den ----
                ctxT = bigp.tile([128, CB, N], MM, tag="big")
                for g in range(NCH):
                    ps_c = [psp.tile([128, 512], dt.float32, tag="pp",
                                     name=f"ps_c{cb_}")
                            for cb_ in range(CB)]
                    for bq in range(B):  # 4 d-steps per kv batch slab
                        vf = streamp.tile([128, 4, 512], MM, tag="vf", bufs=2)
                        nc.sync.dma_start(
                            vf[:],
                            vT_dram[bq, :, g * 512:(g + 1) * 512].rearrange(
                                "(j p) n -> p j n", p=128
                            ),
                        )
                        for j in range(4):
                            ds = bq * 4 + j
                            for cb in range(CB):
                                nc.tensor.matmul(
                                    ps_c[cb][:],
                                    aT[:, ds, cb * 128:(cb + 1) * 128],
                                    vf[:, j, :],
                                    start=(ds == 0),
                                    stop=(ds == NB - 1),
                                )
                    for cb in range(CB):
                        nc.scalar.activation(
                            ctxT[:, cb, g * 512:(g + 1) * 512],
                            ps_c[cb][:],
                            AF.Copy,
                            scale=inv_den[:, cb:cb + 1],
                        )

                # ---- outT = Wout^T @ ctx^T  (out returned transposed; host
                # flips [C, N] -> [N, C]) ----
                for g in range(NCH):
                    po = psp.tile([128, 512], dt.float32, tag="pp")
                    for cb in range(CB):
                        nc.tensor.matmul(
                            po[0:C, :],
                            Wout_r[:, cb, :],
                            ctxT[:, cb, g * 512:(g + 1) * 512],
                            start=(cb == 0),
                            stop=(cb == CB - 1),
                        )
                    ot = streamp.tile([C, 512], dt.float32, tag="ot")
                    nc.vector.tensor_copy(out=ot[:], in_=po[0:C, :])
                    nc.sync.dma_start(
                        out_d[inst, :, g * 512:(g + 1) * 512], ot[:]
                    )

    nc.compile()
    return nc


def _get_nc():
    global _nc
    if _nc is None:
        _nc = _build()
    return _nc


def kernel(emb, Wq, Wk, Wv, Wout):
    emb = np.ascontiguousarray(emb, dtype=np.float32)
    Wq = np.ascontiguousarray(Wq, dtype=np.float32)
    Wk = np.ascontiguousarray(Wk, dtype=np.float32)
    Wv = np.ascontiguousarray(Wv, dtype=np.float32)
    Wout = np.ascontiguousarray(Wout, dtype=np.float32)
    emb_l, emb_u = emb[:B], emb[B:]
    ident = np.eye(128, dtype=np.float32)
    ones = np.ones((128, 128), dtype=np.float32)

    in_maps = []
    for core in range(8):
        if core < 4:
            qb, kvb = emb_l[2 * core:2 * core + 2], emb_u
        else:
            j = core - 4
            qb, kvb = emb_u[2 * j:2 * j + 2], emb_l
        in_maps.append({
            "embq": np.ascontiguousarray(qb), "embkv": np.ascontiguousarray(kvb),
            "Wq": Wq, "Wk": Wk, "Wv": Wv, "Wout": Wout, "ident": ident,
            "ones": ones,
        })

    res = run_bass_kernel_spmd(_get_nc(), in_maps, list(range(8))).results

    out = np.empty((2 * B, N, C), np.float32)
    for core in range(8):
        o = res[core]["out"].transpose(0, 2, 1)  # [2, C, N] -> [2, N, C]
        if core < 4:
            out[2 * core:2 * core + 2] = o
        else:
            j = core - 4
            out[B + 2 * j:B + 2 * j + 2] = o
    return out



# revision 5
# speedup vs baseline: 1.4447x; 1.4447x over previous
"""Cross-attention (global, batch-flattened K/V) Trainium2 kernel, v3.

Problem: emb [16, 4096, 64]; two cross-attention halves:
  out_l2u = cross(q=emb[:8],  kv=emb[8:])   -> rows 0..7
  out_u2l = cross(q=emb[8:],  kv=emb[:8])   -> rows 8..15
cross(): q/k/v proj (64->512), s = einsum('bnc,nd->bcd', q, kflat),
InstanceNorm over (CH, B*CH) plane per b, softmax over d, ctx = a @ vflat^T,
out = ctx @ Wout.

Sharding: 16 independent (cross, q-batch) instances, 2 per core.
Cores 0-3: q from lower half (kv = upper), cores 4-7: q from upper
(kv = lower). No collectives; weights replicated.

v3 design: bf16 matmul operands everywhere (tolerance 2e-2); s computed
TRANSPOSED (sT[d,c] = kf^T @ q) so the exp'd sT is directly the lhsT of
the ctx matmul -- no aT transposes; emb is cast to bf16 and stored
zero-padded [N, 128] in DRAM once, then embT tiles come from XBAR DMA
transpose (no PE transposes at all); vT production fused into instance
0's s-pass; softmax denominator via ones-row matmul + K=1 transposer
matmuls. All n-ordering is natural; no output permutation.
"""

import numpy as np
import concourse.bass as bass
import concourse.mybir as mybir
import concourse.tile as tile
from concourse import bacc
from concourse.bass_utils import run_bass_kernel_spmd

dt = mybir.dt
AF = mybir.ActivationFunctionType
ALU = mybir.AluOpType

B = 8            # batches per half
N = 4096         # sequence length
C = 64           # embedding channels
CH = 512         # num_heads * C
NB = N // 128    # 32 n-blocks
NCH = N // 512   # 8 chunks of 512
CB = CH // 128   # 4 c-blocks
D = B * CH       # 4096 flattened kv dim
EPS = 1e-5
BF = dt.bfloat16
F32R = dt.float32r
PLANE = float(CH * D)  # InstanceNorm plane size per instance

_nc = None


def _build():
    nc = bacc.Bacc("TRN2", target_bir_lowering=False, debug=False, num_devices=8)

    embq = nc.declare_dram_parameter("embq", [2, N, C], dt.float32, isOutput=False)
    embkv = nc.declare_dram_parameter("embkv", [B, N, C], dt.float32, isOutput=False)
    Wq_d = nc.declare_dram_parameter("Wq", [C, CH], dt.float32, isOutput=False)
    Wk_d = nc.declare_dram_parameter("Wk", [C, CH], dt.float32, isOutput=False)
    Wv_d = nc.declare_dram_parameter("Wv", [C, CH], dt.float32, isOutput=False)
    Wout_d = nc.declare_dram_parameter("Wout", [CH, C], dt.float32, isOutput=False)
    ident_d = nc.declare_dram_parameter("ident", [128, 128], dt.float32, isOutput=False)
    ones_d = nc.declare_dram_parameter("ones", [128, 128], dt.float32, isOutput=False)
    out_d = nc.declare_dram_parameter("out", [2, C, N], dt.float32, isOutput=True)

    # bf16 emb, zero-padded channel dim to 128 for XBAR transpose loads.
    # batches 0..7 = kv half, 8..9 = the two q batches.
    emb_bf = nc.dram_tensor("emb_bf", [B + 2, N, 128], BF)
    # vT scratch: [db, g, p(ch sub), j(ch block), n within chunk] bf16
    vT_dram = nc.dram_tensor("vT_scratch", [B, NCH, 128, CB, 512], BF)

    with tile.TileContext(nc) as tc:
        with (
            tc.tile_pool(name="const", bufs=1) as constp,
            tc.tile_pool(name="io", bufs=2) as iop,
            tc.tile_pool(name="embt", bufs=2) as embtp,
            tc.tile_pool(name="res", bufs=1) as resp,
            tc.tile_pool(name="stream", bufs=2) as streamp,
            tc.tile_pool(name="small", bufs=1) as smallp,
            tc.tile_pool(name="ps", bufs=8, space="PSUM") as psp,
        ):
            # ---- constants ----
            ones_f = iop.tile([128, 128], dt.float32, tag="wst")
            nc.sync.dma_start(ones_f[:], ones_d[:])
            ones_r = constp.tile([128, 128], F32R, tag="ones_r")
            nc.vector.tensor_copy(out=ones_r[:], in_=ones_f[:])
            onescol = constp.tile([128, 1], BF, tag="onescol")
            nc.vector.tensor_copy(out=onescol[:], in_=ones_f[:, 0:1])

            w_bf = {}
            for name, wd in (("Wq", Wq_d), ("Wk", Wk_d), ("Wv", Wv_d)):
                wst = iop.tile([C, CH], dt.float32, tag="wst")
                nc.sync.dma_start(wst[:], wd[:])
                wb = constp.tile([C, CH], BF, tag=f"{name}_bf")
                nc.vector.tensor_copy(out=wb[:], in_=wst[:])
                w_bf[name] = wb
            Wq_b, Wk_b, Wv_b = w_bf["Wq"], w_bf["Wk"], w_bf["Wv"]

            wost = iop.tile([128, CB, C], dt.float32, tag="wst")
            nc.sync.dma_start(
                wost[:], Wout_d[:].rearrange("(cb p) c -> p cb c", p=128)
            )
            Wout_b = constp.tile([128, CB, C], BF, tag="Wout_bf")
            nc.vector.tensor_copy(out=Wout_b[:], in_=wost[:])

            # ---- preamble: cast emb to bf16, store zero-padded to DRAM ----
            def to_bf(src, bslot):
                lt = iop.tile([128, NB, C], dt.float32, tag="ld")
                nc.sync.dma_start(
                    lt[:], src.rearrange("(p nb) c -> p nb c", p=128)
                )
                lb = iop.tile([128, NB, 128], BF, tag="ldb")
                nc.vector.memset(lb[:, :, C:128], 0.0)
                nc.vector.tensor_copy(out=lb[:, :, 0:C], in_=lt[:])
                nc.sync.dma_start(
                    emb_bf[bslot].rearrange("(p nb) c -> p nb c", p=128),
                    lb[:],
                )

            to_bf(embq[0], B)
            for db in range(B):
                to_bf(embkv[db], db)
            to_bf(embq[1], B + 1)

            def load_embt(bslot):
                et = embtp.tile([128, N], BF, tag="embt")
                nc.sync.dma_start_transpose(et[:], emb_bf[bslot])
                return et

            # ---- persistent SBUF tensors ----
            q_sb = resp.tile([128, NB, CH], BF, tag="q")      # 32KB/part
            kf_sb = resp.tile([128, NB, CH], BF, tag="kf")    # 32KB/part
            sT = resp.tile([128, NB, CH], BF, tag="sT")       # 32KB/part

            # ---- helper: project [64,N] embT against W -> [128, NB, CH] ----
            def proj(et, w, dest, eng):
                for nb in range(NB):
                    pt = psp.tile([128, 512], dt.float32, tag="pp")
                    nc.tensor.matmul(
                        pt[:],
                        et[0:C, nb * 128:(nb + 1) * 128],
                        w[:],
                        start=True,
                        stop=True,
                    )
                    if eng == "v":
                        nc.vector.tensor_copy(out=dest[:, nb, :], in_=pt[:])
                    else:
                        nc.scalar.activation(dest[:, nb, :], pt[:], AF.Copy)

            # ---- per-instance state tiles ----
            ssum = smallp.tile([128, 2, NB], dt.float32, tag="ssum")
            ssq = smallp.tile([128, 2, NB], dt.float32, tag="ssq")
            stats2 = smallp.tile([128, 2, 8], dt.float32, tag="stats2")
            invden2 = smallp.tile([128, 2, CB], dt.float32, tag="invden2")

            def s_pass(inst, first):
                """Accumulate sT for instance `inst`; when first, also
                produce vT to DRAM."""
                for db in range(B):
                    et = load_embt(db)
                    if first:
                        # vT for this batch -> DRAM
                        for cb in range(CB):
                            for g in range(NCH):
                                pt = psp.tile([128, 512], dt.float32, tag="pp")
                                nc.tensor.matmul(
                                    pt[:],
                                    Wv_b[:, cb * 128:(cb + 1) * 128],
                                    et[0:C, g * 512:(g + 1) * 512],
                                    start=True,
                                    stop=True,
                                )
                                st = streamp.tile([128, 512], BF, tag="vtst",
                                                  bufs=3)
                                nc.scalar.activation(st[:], pt[:], AF.Copy)
                                nc.sync.dma_start(
                                    vT_dram[db, g, :, cb, :], st[:]
                                )
                    # kf projection for this batch
                    proj(et, Wk_b, kf_sb, "v")
                    # sT accumulation: 4 banks over d-chunks
                    ps_sT = [psp.tile([128, 512], dt.float32, tag="pp",
                                      name=f"ps_sT{i}") for i in range(CB)]
                    for nb in range(NB):
                        for dc in range(CB):
                            nc.tensor.matmul(
                                ps_sT[dc][:],
                                kf_sb[:, nb, dc * 128:(dc + 1) * 128],
                                q_sb[:, nb, :],
                                start=(nb == 0),
                                stop=(nb == NB - 1),
                            )
                    for dc in range(CB):
                        kb = db * CB + dc
                        nc.scalar.activation(
                            sT[:, kb, :], ps_sT[dc][:], AF.Copy,
                            accum_out=ssum[:, inst, kb:kb + 1],
                        )
                        nc.scalar.activation(
                            ps_sT[dc][:], ps_sT[dc][:], AF.Square,
                            accum_out=ssq[:, inst, kb:kb + 1],
                        )

            def stats_exp_den(inst):
                """InstanceNorm stats -> exp in place -> inv softmax denom."""
                stats = stats2[:, inst, :]
                red = smallp.tile([128, 2], dt.float32, tag="red", bufs=2)
                nc.vector.tensor_reduce(
                    out=red[:, 0:1], in_=ssum[:, inst, :],
                    axis=mybir.AxisListType.X, op=ALU.add,
                )
                nc.vector.tensor_reduce(
                    out=red[:, 1:2], in_=ssq[:, inst, :],
                    axis=mybir.AxisListType.X, op=ALU.add,
                )
                red_r = smallp.tile([128, 2], F32R, tag="red_r", bufs=2)
                nc.vector.tensor_copy(out=red_r[:], in_=red[:])
                ptr = psp.tile([128, 512], dt.float32, tag="pp")
                nc.tensor.matmul(
                    ptr[:, 0:2], ones_r[:], red_r[:], start=True, stop=True
                )
                nc.scalar.activation(
                    stats[:, 0:2], ptr[:, 0:2], AF.Copy, bias=0.0,
                    scale=1.0 / PLANE,
                )
                mu = stats[:, 0:1]
                ex2 = stats[:, 1:2]
                musq = stats[:, 2:3]
                var = stats[:, 3:4]
                std = stats[:, 4:5]
                rstd = stats[:, 5:6]
                nmr = stats[:, 6:7]
                nc.vector.tensor_tensor(out=musq, in0=mu, in1=mu, op=ALU.mult)
                nc.vector.tensor_tensor(out=var, in0=ex2, in1=musq,
                                        op=ALU.subtract)
                nc.vector.tensor_scalar_add(var, var, EPS)
                nc.scalar.activation(std, var, AF.Sqrt, bias=0.0)
                nc.vector.reciprocal(rstd, std)
                nc.vector.tensor_tensor(out=nmr, in0=mu, in1=rstd, op=ALU.mult)
                nc.scalar.mul(nmr, nmr, -1.0)

                # exp((s - mu) * rstd) in place on sT (bf16)
                for kb in range(NB):
                    nc.scalar.activation(
                        sT[:, kb, :], sT[:, kb, :], AF.Exp,
                        bias=nmr, scale=rstd,
                    )
                # den row [1, CH] via ones-column matmul over all d blocks
                ps_den = psp.tile([128, 512], dt.float32, tag="pp",
                                  name="ps_den")
                for kb in range(NB):
                    nc.tensor.matmul(
                        ps_den[0:1, :], onescol[:], sT[:, kb, :],
                        start=(kb == 0), stop=(kb == NB - 1),
                    )
                den_sb = smallp.tile([1, CH], BF, tag="den_sb", bufs=2)
                nc.vector.tensor_copy(out=den_sb[:], in_=ps_den[0:1, :])
                # transpose den row -> [128, CB] via K=1 matmuls
                ptd = psp.tile([128, 512], dt.float32, tag="pp", name="ptd")
                for cb in range(CB):
                    nc.tensor.matmul(
                        ptd[:, cb:cb + 1],
                        den_sb[0:1, cb * 128:(cb + 1) * 128],
                        onescol[0:1, 0:1],
                        start=True, stop=True,
                    )
                denT = smallp.tile([128, CB], dt.float32, tag="denT", bufs=2)
                nc.vector.tensor_copy(out=denT[:], in_=ptd[:, 0:CB])
                nc.vector.reciprocal(invden2[:, inst, :], denT[:])

            def ctx_out(inst):
                inv_den = invden2[:, inst, :]
                for g in range(NCH):
                    ps_ctx = [psp.tile([128, 512], dt.float32, tag="pp",
                                       name=f"ps_ctx{i}") for i in range(CB)]
                    for db in range(B):
                        vf = streamp.tile([128, CB, 512], BF, tag="vf", bufs=3)
                        nc.sync.dma_start(vf[:], vT_dram[db, g])
                        for j in range(CB):
                            ds = db * CB + j
                            for cb in range(CB):
                                nc.tensor.matmul(
                                    ps_ctx[cb][:],
                                    sT[:, ds, cb * 128:(cb + 1) * 128],
                                    vf[:, j, :],
                                    start=(ds == 0),
                                    stop=(ds == NB - 1),
                                )
                    ctxg = streamp.tile([128, CB, 512], BF, tag="ctxg", bufs=2)
                    for cb in range(CB):
                        nc.scalar.activation(
                            ctxg[:, cb, :], ps_ctx[cb][:], AF.Copy,
                            scale=inv_den[:, cb:cb + 1],
                        )
                    po = psp.tile([128, 512], dt.float32, tag="pp", name="po")
                    for cb in range(CB):
                        nc.tensor.matmul(
                            po[0:C, :],
                            Wout_b[:, cb, :],
                            ctxg[:, cb, :],
                            start=(cb == 0),
                            stop=(cb == CB - 1),
                        )
                    ot = streamp.tile([C, 512], dt.float32, tag="ot", bufs=2)
                    nc.vector.tensor_copy(out=ot[:], in_=po[0:C, :])
                    nc.sync.dma_start(
                        out_d[inst, :, g * 512:(g + 1) * 512], ot[:]
                    )

            # ================= schedule =================
            et_q0 = load_embt(B)
            proj(et_q0, Wq_b, q_sb, "v")
            s_pass(0, first=True)
            # filler work for the exp(0) gap: build instance 1's q
            et_q1 = load_embt(B + 1)
            proj(et_q1, Wq_b, q_sb, "v")
            stats_exp_den(0)
            ctx_out(0)
            # instance 1
            s_pass(1, first=False)
            stats_exp_den(1)
            ctx_out(1)

    nc.compile()
    return nc


def _get_nc():
    global _nc
    if _nc is None:
        _nc = _build()
    return _nc


def kernel(emb, Wq, Wk, Wv, Wout):
    emb = np.ascontiguousarray(emb, dtype=np.float32)
    Wq = np.ascontiguousarray(Wq, dtype=np.float32)
    Wk = np.ascontiguousarray(Wk, dtype=np.float32)
    Wv = np.ascontiguousarray(Wv, dtype=np.float32)
    Wout = np.ascontiguousarray(Wout, dtype=np.float32)
    emb_l, emb_u = emb[:B], emb[B:]
    ident = np.eye(128, dtype=np.float32)
    ones = np.ones((128, 128), dtype=np.float32)

    in_maps = []
    for core in range(8):
        if core < 4:
            qb, kvb = emb_l[2 * core:2 * core + 2], emb_u
        else:
            j = core - 4
            qb, kvb = emb_u[2 * j:2 * j + 2], emb_l
        in_maps.append({
            "embq": np.ascontiguousarray(qb), "embkv": np.ascontiguousarray(kvb),
            "Wq": Wq, "Wk": Wk, "Wv": Wv, "Wout": Wout, "ident": ident,
            "ones": ones,
        })

    res = run_bass_kernel_spmd(_get_nc(), in_maps, list(range(8))).results

    out = np.empty((2 * B, N, C), np.float32)
    for core in range(8):
        o = res[core]["out"].transpose(0, 2, 1)  # [2, C, N] -> [2, N, C]
        if core < 4:
            out[2 * core:2 * core + 2] = o
        else:
            j = core - 4
            out[B + 2 * j:B + 2 * j + 2] = o
    return out


# revision 9
# speedup vs baseline: 6.1665x; 4.2684x over previous
"""Cross-attention (global, batch-flattened K/V) Trainium2 kernel, v4.

Problem: emb [16, 4096, 64]; two cross-attention halves:
  out_l2u = cross(q=emb[:8],  kv=emb[8:])   -> rows 0..7
  out_u2l = cross(q=emb[8:],  kv=emb[:8])   -> rows 8..15
cross(): q/k/v proj (64->512), s = einsum('bnc,nd->bcd', q, kflat),
InstanceNorm over (CH, B*CH) plane per b, softmax over d, ctx = a @ vflat^T,
out = ctx @ Wout.

Sharding: 16 independent (cross, q-batch) instances, 2 per core.
Cores 0-3: q from lower half (kv = upper), cores 4-7: q from upper
(kv = lower). No collectives; weights replicated.

v4 key insight: the score matrix is RANK-64 (all projections factor
through the 64-channel embedding), so both big GEMMs contract through
64-dim intermediates:
  sT[d,:] for kv batch db = Wk^T @ (emb_db^T @ q)      (m = emb^T q: [64,512])
  out = sum_db emb_db @ ((Wv @ aT_db / den) @ Wout)    ([64,64] per db)
This cuts per-core matmul work ~7-30x vs materialized kf/vf GEMMs. The
(CH x D) score plane is still materialized (transposed, bf16) for
InstanceNorm + softmax, drained with fused stats. No vT/kf scratch at
all. emb is cast to bf16 once, stored pair-packed+padded [N,128] in
DRAM; natural-order loads feed the m phase, XBAR DMA-transposed loads
feed the output phase.
"""

import numpy as np
import concourse.bass as bass
import concourse.mybir as mybir
import concourse.tile as tile
from concourse import bacc
from concourse.bass_utils import run_bass_kernel_spmd

dt = mybir.dt
AF = mybir.ActivationFunctionType
ALU = mybir.AluOpType

B = 8            # batches per half
N = 4096         # sequence length
C = 64           # embedding channels
CH = 512         # num_heads * C
NB = N // 128    # 32 n-blocks
CB = CH // 128   # 4 c-blocks
D = B * CH       # 4096 flattened kv dim
EPS = 1e-5
BF = dt.bfloat16
PLANE = float(CH * D)  # InstanceNorm plane size per instance

_nc = None


def _build():
    nc = bacc.Bacc("TRN2", target_bir_lowering=False, debug=False, num_devices=8)

    embq = nc.declare_dram_parameter("embq", [2, N, C], dt.float32, isOutput=False)
    embkv = nc.declare_dram_parameter("embkv", [B, N, C], dt.float32, isOutput=False)
    Wq_d = nc.declare_dram_parameter("Wq", [C, CH], dt.float32, isOutput=False)
    Wk_d = nc.declare_dram_parameter("Wk", [C, CH], dt.float32, isOutput=False)
    Wv_d = nc.declare_dram_parameter("Wv", [C, CH], dt.float32, isOutput=False)
    Wout_d = nc.declare_dram_parameter("Wout", [CH, C], dt.float32, isOutput=False)
    ident_d = nc.declare_dram_parameter("ident", [128, 128], dt.float32, isOutput=False)
    ones_d = nc.declare_dram_parameter("ones", [128, 128], dt.float32, isOutput=False)
    out_d = nc.declare_dram_parameter("out", [2, N, C], dt.float32, isOutput=True)

    # bf16 emb, pair-packed: slot k<4 = kv batches (2k | 2k+1) in cols
    # (0:64 | 64:128); slot 4 = (q0 | q1).
    emb_bf = nc.dram_tensor("emb_bf", [5, N, 128], BF)

    with tile.TileContext(nc) as tc:
        with (
            tc.tile_pool(name="const", bufs=1) as constp,
            tc.tile_pool(name="io", bufs=2) as iop,
            tc.tile_pool(name="res", bufs=1) as resp,
            tc.tile_pool(name="stream", bufs=2) as streamp,
            tc.tile_pool(name="small", bufs=1) as smallp,
            tc.tile_pool(name="ps", bufs=6, space="PSUM") as psp,
        ):
            # ---- constants ----
            ident = constp.tile([128, 128], dt.float32, tag="ident")
            nc.sync.dma_start(ident[:], ident_d[:])
            ident_bf = constp.tile([128, 128], BF, tag="ident_bf")
            nc.vector.tensor_copy(out=ident_bf[:], in_=ident[:])
            ones_f = iop.tile([128, 128], dt.float32, tag="wst")
            nc.sync.dma_start(ones_f[:], ones_d[:])
            ones_r = constp.tile([128, 128], dt.float32r, tag="ones_r")
            nc.vector.tensor_copy(out=ones_r[:], in_=ones_f[:])
            onescol = constp.tile([128, 1], BF, tag="onescol")
            nc.vector.tensor_copy(out=onescol[:], in_=ones_f[:, 0:1])

            w_bf = {}
            for name, wd in (("Wq", Wq_d), ("Wk", Wk_d), ("Wv", Wv_d)):
                wst = iop.tile([C, CH], dt.float32, tag="wst")
                nc.sync.dma_start(wst[:], wd[:])
                wb = constp.tile([C, CH], BF, tag=f"{name}_bf")
                nc.vector.tensor_copy(out=wb[:], in_=wst[:])
                if name == "Wv":
                    wv_f32 = wst
                w_bf[name] = wb
            Wq_b, Wk_b = w_bf["Wq"], w_bf["Wk"]

            # WvT [128(ch sub), CB, 64(c')] via 4 fp32 PE transposes
            WvT_b = constp.tile([128, CB, C], BF, tag="WvT_bf")
            ptw = psp.tile([128, 512], dt.float32, tag="pp")
            for k in range(CB):
                nc.tensor.transpose(
                    ptw[:, k * 128:k * 128 + C],
                    wv_f32[:, k * 128:(k + 1) * 128],
                    ident[0:C, 0:C],
                )
            for k in range(CB):
                nc.vector.tensor_copy(
                    out=WvT_b[:, k, :], in_=ptw[:, k * 128:k * 128 + C]
                )

            wost = iop.tile([128, CB, C], dt.float32, tag="wst")
            nc.sync.dma_start(
                wost[:], Wout_d[:].rearrange("(cb p) c -> p cb c", p=128)
            )
            Wout_b = constp.tile([128, CB, C], BF, tag="Wout_bf")
            nc.vector.tensor_copy(out=Wout_b[:], in_=wost[:])

            # ---- preamble: cast emb to bf16, store pair-packed to DRAM ----
            def to_bf(slot, srcA, srcB):
                for h in range(2):
                    lb = iop.tile([128, 16, 128], BF, tag="ldb")
                    for half, src in ((0, srcA), (1, srcB)):
                        lt = iop.tile([128, 16, C], dt.float32, tag="ld")
                        nc.sync.dma_start(
                            lt[:],
                            src[h * 2048:(h + 1) * 2048, :].rearrange(
                                "(p nb) c -> p nb c", p=128
                            ),
                        )
                        nc.vector.tensor_copy(
                            out=lb[:, :, half * C:(half + 1) * C], in_=lt[:]
                        )
                    nc.sync.dma_start(
                        emb_bf[slot, h * 2048:(h + 1) * 2048, :].rearrange(
                            "(p nb) c -> p nb c", p=128
                        ),
                        lb[:],
                    )

            for k in range(4):
                to_bf(k, embkv[2 * k], embkv[2 * k + 1])
            to_bf(4, embq[0], embq[1])

            # ---- persistent SBUF tensors ----
            q_sb = resp.tile([128, NB, CH], BF, tag="q")      # 32KB/part

            ssum = smallp.tile([128, 2, NB], dt.float32, tag="ssum")
            ssq = smallp.tile([128, 2, NB], dt.float32, tag="ssq")
            stats2 = smallp.tile([128, 2, 8], dt.float32, tag="stats2")
            invden2 = smallp.tile([128, 2, CB], dt.float32, tag="invden2")

            def build_q(inst):
                """q[n,ch] rows n = p*32+nb, matching the m-phase eb rows."""
                eb_q = streamp.tile([128, NB, 128], BF, tag="eb")
                nc.sync.dma_start(
                    eb_q[:], emb_bf[4].rearrange("(p nb) c -> p nb c", p=128)
                )
                coff = inst * C
                embt_q = streamp.tile([C, N], BF, tag="embt", bufs=1)
                for grp in range(8):
                    pb = psp.tile([128, 512], BF, tag="ppb", bufs=2)
                    for j4 in range(4):
                        nb = grp * 4 + j4
                        nc.tensor.transpose(
                            pb[0:C, j4 * 128:(j4 + 1) * 128],
                            eb_q[:, nb, coff:coff + C],
                            ident_bf[:],
                        )
                    nc.vector.tensor_copy(
                        out=embt_q[:, grp * 512:(grp + 1) * 512],
                        in_=pb[0:C, :],
                    )
                for nb in range(NB):
                    pt = psp.tile([128, 512], dt.float32, tag="pp")
                    nc.tensor.matmul(
                        pt[:],
                        embt_q[:, nb * 128:(nb + 1) * 128],
                        Wq_b[:],
                        start=True,
                        stop=True,
                    )
                    nc.vector.tensor_copy(out=q_sb[:, nb, :], in_=pt[:])

            def s_pass(inst):
                """sT[d, c] = Wk^T @ (emb_db^T @ q) per kv batch, with fused
                InstanceNorm stats on the drains."""
                sT = resp.tile([128, NB, CH], BF, tag="sT", bufs=2)
                eb = None
                for db in range(B):
                    if db % 2 == 0:
                        eb = streamp.tile([128, NB, 128], BF, tag="eb")
                        nc.sync.dma_start(
                            eb[:],
                            emb_bf[db // 2].rearrange(
                                "(p nb) c -> p nb c", p=128
                            ),
                        )
                    coff = (db % 2) * C
                    pm = psp.tile([128, 512], dt.float32, tag="pp", name="pm")
                    for nb in range(NB):
                        nc.tensor.matmul(
                            pm[0:C, :],
                            eb[:, nb, coff:coff + C],
                            q_sb[:, nb, :],
                            start=(nb == 0),
                            stop=(nb == NB - 1),
                        )
                    m_sb = streamp.tile([C, CH], BF, tag="msb")
                    nc.vector.tensor_copy(out=m_sb[:], in_=pm[0:C, :])
                    ps_sT = [psp.tile([128, 512], dt.float32, tag="pp",
                                      name=f"ps_sT{i}") for i in range(CB)]
                    for dc in range(CB):
                        nc.tensor.matmul(
                            ps_sT[dc][:],
                            Wk_b[:, dc * 128:(dc + 1) * 128],
                            m_sb[:],
                            start=True,
                            stop=True,
                        )
                    for dc in range(CB):
                        kb = db * CB + dc
                        nc.scalar.activation(
                            sT[:, kb, :], ps_sT[dc][:], AF.Copy,
                            accum_out=ssum[:, inst, kb:kb + 1],
                        )
                        nc.scalar.activation(
                            ps_sT[dc][:], ps_sT[dc][:], AF.Square,
                            accum_out=ssq[:, inst, kb:kb + 1],
                        )
                return sT

            def stats_exp(inst, sT):
                """InstanceNorm stats -> exp in place on sT (scalar engine)."""
                stats = stats2[:, inst, :]
                red = smallp.tile([128, 2], dt.float32, tag="red", bufs=2)
                nc.vector.tensor_reduce(
                    out=red[:, 0:1], in_=ssum[:, inst, :],
                    axis=mybir.AxisListType.X, op=ALU.add,
                )
                nc.vector.tensor_reduce(
                    out=red[:, 1:2], in_=ssq[:, inst, :],
                    axis=mybir.AxisListType.X, op=ALU.add,
                )
                red_r = smallp.tile([128, 2], dt.float32r, tag="red_r", bufs=2)
                nc.vector.tensor_copy(out=red_r[:], in_=red[:])
                ptr = psp.tile([128, 512], dt.float32, tag="pp")
                nc.tensor.matmul(
                    ptr[:, 0:2], ones_r[:], red_r[:], start=True, stop=True
                )
                nc.scalar.activation(
                    stats[:, 0:2], ptr[:, 0:2], AF.Copy, bias=0.0,
                    scale=1.0 / PLANE,
                )
                mu = stats[:, 0:1]
                ex2 = stats[:, 1:2]
                musq = stats[:, 2:3]
                var = stats[:, 3:4]
                std = stats[:, 4:5]
                rstd = stats[:, 5:6]
                nmr = stats[:, 6:7]
                nc.vector.tensor_tensor(out=musq, in0=mu, in1=mu, op=ALU.mult)
                nc.vector.tensor_tensor(out=var, in0=ex2, in1=musq,
                                        op=ALU.subtract)
                nc.vector.tensor_scalar_add(var, var, EPS)
                nc.scalar.activation(std, var, AF.Sqrt, bias=0.0)
                nc.vector.reciprocal(rstd, std)
                nc.vector.tensor_tensor(out=nmr, in0=mu, in1=rstd, op=ALU.mult)
                nc.scalar.mul(nmr, nmr, -1.0)
                for kb in range(NB):
                    nc.scalar.activation(
                        sT[:, kb, :], sT[:, kb, :], AF.Exp,
                        bias=nmr, scale=rstd,
                    )

            def den_pass(inst, sT):
                """softmax denominator per ch -> invden2[:, inst, :]."""
                ps_den = psp.tile([128, 512], dt.float32, tag="pp",
                                  name="ps_den")
                for kb in range(NB):
                    nc.tensor.matmul(
                        ps_den[0:1, :], onescol[:], sT[:, kb, :],
                        start=(kb == 0), stop=(kb == NB - 1),
                    )
                den_sb = smallp.tile([1, CH], BF, tag="den_sb", bufs=2)
                nc.vector.tensor_copy(out=den_sb[:], in_=ps_den[0:1, :])
                ptd = psp.tile([128, 512], dt.float32, tag="pp", name="ptd")
                for cb in range(CB):
                    nc.tensor.matmul(
                        ptd[:, cb:cb + 1],
                        den_sb[0:1, cb * 128:(cb + 1) * 128],
                        onescol[0:1, 0:1],
                        start=True, stop=True,
                    )
                denT = smallp.tile([128, CB], dt.float32, tag="denT", bufs=2)
                nc.vector.tensor_copy(out=denT[:], in_=ptd[:, 0:CB])
                nc.vector.reciprocal(invden2[:, inst, :], denT[:])

            etp_tiles = []

            def ctx_out(inst, sT):
                """out = sum_db emb_db @ ((Wv @ aT_db / den) @ Wout)."""
                inv_den = invden2[:, inst, :]
                if not etp_tiles:
                    for k in range(4):
                        etp = streamp.tile([128, N], BF, tag="etp", bufs=4)
                        nc.sync.dma_start_transpose(etp[:], emb_bf[k])
                        etp_tiles.append(etp)
                pwo = psp.tile([128, 512], dt.float32, tag="pp", name="pwo")
                for db in range(B):
                    pwa = psp.tile([128, 512], dt.float32, tag="pp",
                                   name="pwa")
                    for chb in range(CB):
                        for j in range(CB):
                            nc.tensor.matmul(
                                pwa[:, chb * C:(chb + 1) * C],
                                sT[:, db * CB + j,
                                   chb * 128:(chb + 1) * 128],
                                WvT_b[:, j, :],
                                start=(j == 0),
                                stop=(j == CB - 1),
                            )
                    wva = streamp.tile([128, CB, C], BF, tag="wva")
                    for chb in range(CB):
                        nc.vector.tensor_scalar_mul(
                            wva[:, chb, :],
                            pwa[:, chb * C:(chb + 1) * C],
                            inv_den[:, chb:chb + 1],
                        )
                    for chb in range(CB):
                        nc.tensor.matmul(
                            pwo[0:C, db * C:(db + 1) * C],
                            wva[:, chb, :],
                            Wout_b[:, chb, :],
                            start=(chb == 0),
                            stop=(chb == CB - 1),
                        )
                # pair-stack wvo: [128(2 batches' c'), 4(pair), 64]
                wvo2 = streamp.tile([128, CB, C], BF, tag="wvo2")
                for db in range(B):
                    nc.vector.tensor_copy(
                        out=wvo2[(db % 2) * C:(db % 2 + 1) * C, db // 2, :],
                        in_=pwo[0:C, db * C:(db + 1) * C],
                    )
                # out[n, c] = sum_pairs embT_pair^T @ wvo_pair
                for nbg in range(4):
                    pout = psp.tile([128, 512], dt.float32, tag="pp",
                                    name="pout")
                    for nb8 in range(8):
                        nb = nbg * 8 + nb8
                        for k in range(4):
                            nc.tensor.matmul(
                                pout[:, nb8 * C:(nb8 + 1) * C],
                                etp_tiles[k][:, nb * 128:(nb + 1) * 128],
                                wvo2[:, k, :],
                                start=(k == 0),
                                stop=(k == 3),
                            )
                    ot = streamp.tile([128, 8, C], dt.float32, tag="ot")
                    nc.vector.tensor_copy(
                        out=ot[:].rearrange("p a b -> p (a b)"), in_=pout[:]
                    )
                    nc.sync.dma_start(
                        out_d[inst, nbg * 1024:(nbg + 1) * 1024, :].rearrange(
                            "(nb p) c -> p nb c", p=128
                        ),
                        ot[:],
                    )

            # ================= schedule =================
            build_q(0)
            sT0 = s_pass(0)
            build_q(1)
            stats_exp(0, sT0)
            sT1 = s_pass(1)
            den_pass(0, sT0)
            stats_exp(1, sT1)
            ctx_out(0, sT0)
            den_pass(1, sT1)
            ctx_out(1, sT1)

    nc.compile()
    return nc


def _get_nc():
    global _nc
    if _nc is None:
        _nc = _build()
    return _nc


def kernel(emb, Wq, Wk, Wv, Wout):
    emb = np.ascontiguousarray(emb, dtype=np.float32)
    Wq = np.ascontiguousarray(Wq, dtype=np.float32)
    Wk = np.ascontiguousarray(Wk, dtype=np.float32)
    Wv = np.ascontiguousarray(Wv, dtype=np.float32)
    Wout = np.ascontiguousarray(Wout, dtype=np.float32)
    emb_l, emb_u = emb[:B], emb[B:]
    ident = np.eye(128, dtype=np.float32)
    ones = np.ones((128, 128), dtype=np.float32)

    in_maps = []
    for core in range(8):
        if core < 4:
            qb, kvb = emb_l[2 * core:2 * core + 2], emb_u
        else:
            j = core - 4
            qb, kvb = emb_u[2 * j:2 * j + 2], emb_l
        in_maps.append({
            "embq": np.ascontiguousarray(qb), "embkv": np.ascontiguousarray(kvb),
            "Wq": Wq, "Wk": Wk, "Wv": Wv, "Wout": Wout, "ident": ident,
            "ones": ones,
        })

    res = run_bass_kernel_spmd(_get_nc(), in_maps, list(range(8))).results

    out = np.empty((2 * B, N, C), np.float32)
    for core in range(8):
        o = res[core]["out"]  # [2, N, C] natural
        if core < 4:
            out[2 * core:2 * core + 2] = o
        else:
            j = core - 4
            out[B + 2 * j:B + 2 * j + 2] = o
    return out


# revision 16
# speedup vs baseline: 7.5659x; 1.2269x over previous
"""Cross-attention (global, batch-flattened K/V) Trainium2 kernel, v5.

Problem: emb [16, 4096, 64]; two cross-attention halves:
  out_l2u = cross(q=emb[:8],  kv=emb[8:])   -> rows 0..7
  out_u2l = cross(q=emb[8:],  kv=emb[:8])   -> rows 8..15
cross(): q/k/v proj (64->512), s = einsum('bnc,nd->bcd', q, kflat),
InstanceNorm over (CH, B*CH) plane per b, softmax over d, ctx = a @ vflat^T,
out = ctx @ Wout.

Sharding: 16 independent (cross, q-batch) instances, 2 per core.
Cores 0-3: q from lower half (kv = upper), cores 4-7: q from upper
(kv = lower). No collectives; weights replicated.

Key insight: the score matrix is RANK-64 (all projections factor
through the 64-channel embedding), so both big GEMMs contract through
64-dim intermediates:
  sT[d,:] for kv batch db = Wk^T @ (emb_db^T @ q)      (m = emb^T q: [64,512])
  outT = sum_db (wvo_db)^T @ emb_db^T,  wvo_db = (Wv @ aT_db / den) @ Wout
The (CH x D) score plane is still materialized (transposed, bf16) for
InstanceNorm + softmax, drained with fused stats (ssum on DVE
tensor_scalar accum, ssq on DVE tensor_tensor_reduce; exp on scalar).
emb is cast to bf16 once, stored pair-packed+padded [N,128] in DRAM;
natural-order loads feed the m phase, XBAR DMA-transposed loads feed
the output phase. q is built straight from the fp32 input via PE
transposes so nothing waits on the DRAM staging. m phase is software-
pipelined (m for batch db+1 issued before sT of batch db).
"""

import numpy as np
import concourse.bass as bass
import concourse.mybir as mybir
import concourse.tile as tile
from concourse import bacc
from concourse.bass_utils import run_bass_kernel_spmd

dt = mybir.dt
AF = mybir.ActivationFunctionType
ALU = mybir.AluOpType

B = 8            # batches per half
N = 4096         # sequence length
C = 64           # embedding channels
CH = 512         # num_heads * C
NB = N // 128    # 32 n-blocks
CB = CH // 128   # 4 c-blocks
D = B * CH       # 4096 flattened kv dim
EPS = 1e-5
BF = dt.bfloat16
PLANE = float(CH * D)  # InstanceNorm plane size per instance

_nc = None


def _build():
    nc = bacc.Bacc("TRN2", target_bir_lowering=False, debug=False, num_devices=8)

    embq = nc.declare_dram_parameter("embq", [2, N, C], dt.float32, isOutput=False)
    embkv = nc.declare_dram_parameter("embkv", [B, N, C], dt.float32, isOutput=False)
    Wq_d = nc.declare_dram_parameter("Wq", [C, CH], dt.float32, isOutput=False)
    Wk_d = nc.declare_dram_parameter("Wk", [C, CH], dt.float32, isOutput=False)
    Wv_d = nc.declare_dram_parameter("Wv", [C, CH], dt.float32, isOutput=False)
    Wout_d = nc.declare_dram_parameter("Wout", [CH, C], dt.float32, isOutput=False)
    ident_d = nc.declare_dram_parameter("ident", [128, 128], dt.float32, isOutput=False)
    ones_d = nc.declare_dram_parameter("ones", [128, 128], dt.float32, isOutput=False)
    out_d = nc.declare_dram_parameter("out", [2, C, N], dt.float32, isOutput=True)

    # bf16 kv emb, pair-packed: slot k = kv batches (2k | 2k+1) in cols
    # (0:64 | 64:128).
    emb_bf = nc.dram_tensor("emb_bf", [4, N, 128], BF)

    with tile.TileContext(nc) as tc:
        with (
            tc.tile_pool(name="const", bufs=1) as constp,
            tc.tile_pool(name="io", bufs=2) as iop,
            tc.tile_pool(name="res", bufs=1) as resp,
            tc.tile_pool(name="stream", bufs=2) as streamp,
            tc.tile_pool(name="small", bufs=1) as smallp,
            tc.tile_pool(name="ps", bufs=6, space="PSUM") as psp,
        ):
            # ---- constants ----
            ident = constp.tile([128, 128], dt.float32, tag="ident")
            nc.sync.dma_start(ident[:], ident_d[:])
            ident_bf = constp.tile([128, 128], BF, tag="ident_bf")
            nc.vector.tensor_copy(out=ident_bf[:], in_=ident[:])
            ones_f = iop.tile([128, 128], dt.float32, tag="wst")
            nc.sync.dma_start(ones_f[:], ones_d[:])
            ones_r = constp.tile([128, 128], dt.float32r, tag="ones_r")
            nc.vector.tensor_copy(out=ones_r[:], in_=ones_f[:])
            onescol = constp.tile([128, 1], BF, tag="onescol")
            nc.vector.tensor_copy(out=onescol[:], in_=ones_f[:, 0:1])

            w_bf = {}
            for name, wd in (("Wq", Wq_d), ("Wk", Wk_d), ("Wv", Wv_d)):
                wst = iop.tile([C, CH], dt.float32, tag="wst")
                nc.sync.dma_start(wst[:], wd[:])
                wb = constp.tile([C, CH], BF, tag=f"{name}_bf")
                nc.vector.tensor_copy(out=wb[:], in_=wst[:])
                if name == "Wv":
                    wv_f32 = wst
                w_bf[name] = wb
            Wq_b, Wk_b = w_bf["Wq"], w_bf["Wk"]

            # WvT [128(ch sub), CB, 64(c')] via 4 fp32 PE transposes
            WvT_b = constp.tile([128, CB, C], BF, tag="WvT_bf")
            ptw = psp.tile([128, 512], dt.float32, tag="pp")
            for k in range(CB):
                nc.tensor.transpose(
                    ptw[:, k * 128:k * 128 + C],
                    wv_f32[:, k * 128:(k + 1) * 128],
                    ident[0:C, 0:C],
                )
            for k in range(CB):
                nc.vector.tensor_copy(
                    out=WvT_b[:, k, :], in_=ptw[:, k * 128:k * 128 + C]
                )

            wost = iop.tile([128, CB, C], dt.float32, tag="wst")
            nc.sync.dma_start(
                wost[:], Wout_d[:].rearrange("(cb p) c -> p cb c", p=128)
            )
            Wout_b = constp.tile([128, CB, C], BF, tag="Wout_bf")
            nc.vector.tensor_copy(out=Wout_b[:], in_=wost[:])

            # ---- preamble: cast kv emb to bf16, pair-packed, to DRAM ----
            def to_bf(slot):
                for h in range(2):
                    lb = iop.tile([128, 16, 128], BF, tag="ldb")
                    for half in range(2):
                        lt = iop.tile([128, 16, C], dt.float32, tag="ld")
                        nc.sync.dma_start(
                            lt[:],
                            embkv[2 * slot + half,
                                  h * 2048:(h + 1) * 2048, :].rearrange(
                                "(p nb) c -> p nb c", p=128
                            ),
                        )
                        nc.vector.tensor_copy(
                            out=lb[:, :, half * C:(half + 1) * C], in_=lt[:]
                        )
                    nc.sync.dma_start(
                        emb_bf[slot, h * 2048:(h + 1) * 2048, :].rearrange(
                            "(p nb) c -> p nb c", p=128
                        ),
                        lb[:],
                    )

            # ---- persistent SBUF tensors ----
            q_sb = resp.tile([128, NB, CH], BF, tag="q")      # 32KB/part

            sq_junk = smallp.tile([128, 512], BF, tag="sq_junk")
            ssum = smallp.tile([128, 2, NB], dt.float32, tag="ssum")
            ssq = smallp.tile([128, 2, NB], dt.float32, tag="ssq")
            stats2 = smallp.tile([128, 2, 8], dt.float32, tag="stats2")
            invden2 = smallp.tile([128, 2, CB], dt.float32, tag="invden2")

            def build_q(inst):
                """q[n,ch] rows n = p*32+nb, straight from fp32 embq."""
                lt = iop.tile([128, NB, C], dt.float32, tag="ld")
                nc.sync.dma_start(
                    lt[:], embq[inst].rearrange("(p nb) c -> p nb c", p=128)
                )
                lbq = iop.tile([128, NB, C], BF, tag="ldb")
                nc.vector.tensor_copy(out=lbq[:], in_=lt[:])
                embt_q = streamp.tile([C, N], BF, tag="embt", bufs=1)
                for grp in range(8):
                    pb = psp.tile([128, 512], BF, tag="ppb", bufs=2)
                    for j4 in range(4):
                        nb = grp * 4 + j4
                        nc.tensor.transpose(
                            pb[0:C, j4 * 128:(j4 + 1) * 128],
                            lbq[:, nb, :],
                            ident_bf[:],
                        )
                    nc.vector.tensor_copy(
                        out=embt_q[:, grp * 512:(grp + 1) * 512],
                        in_=pb[0:C, :],
                    )
                for nb in range(NB):
                    pt = psp.tile([128, 512], dt.float32, tag="pp")
                    nc.tensor.matmul(
                        pt[:],
                        embt_q[:, nb * 128:(nb + 1) * 128],
                        Wq_b[:],
                        start=True,
                        stop=True,
                    )
                    nc.vector.tensor_copy(out=q_sb[:, nb, :], in_=pt[:])

            def load_eb(pair):
                eb = streamp.tile([128, NB, 128], BF, tag="eb")
                nc.sync.dma_start(
                    eb[:],
                    emb_bf[pair].rearrange("(p nb) c -> p nb c", p=128),
                )
                return eb

            def m_phase(db, eb):
                coff = (db % 2) * C
                pm = psp.tile([128, 512], dt.float32, tag="pp", name="pm")
                for nb in range(NB):
                    nc.tensor.matmul(
                        pm[0:C, :],
                        eb[:, nb, coff:coff + C],
                        q_sb[:, nb, :],
                        start=(nb == 0),
                        stop=(nb == NB - 1),
                    )
                m_sb = streamp.tile([C, CH], BF, tag="msb")
                nc.vector.tensor_copy(out=m_sb[:], in_=pm[0:C, :])
                return m_sb

            def sT_phase(inst, db, m_sb, sT):
                ps_sT = [psp.tile([128, 512], dt.float32, tag="pp",
                                  name=f"ps_sT{i}") for i in range(CB)]
                for dc in range(CB):
                    nc.tensor.matmul(
                        ps_sT[dc][:],
                        Wk_b[:, dc * 128:(dc + 1) * 128],
                        m_sb[:],
                        start=True,
                        stop=True,
                    )
                for dc in range(CB):
                    kb = db * CB + dc
                    nc.scalar.activation(
                        sT[:, kb, :], ps_sT[dc][:], AF.Copy,
                        accum_out=ssum[:, inst, kb:kb + 1],
                    )
                    nc.scalar.activation(
                        ps_sT[dc][:], ps_sT[dc][:], AF.Square,
                        accum_out=ssq[:, inst, kb:kb + 1],
                    )

            def s_pass(inst, ebs):
                """sT[d, c] = Wk^T @ (emb_db^T @ q), software-pipelined."""
                sT = resp.tile([128, NB, CH], BF, tag="sT", bufs=2)
                prev = None
                for db in range(B):
                    if db % 2 == 0 and ebs[db // 2] is None:
                        ebs[db // 2] = load_eb(db // 2)
                    m_sb = m_phase(db, ebs[db // 2])
                    if prev is not None:
                        sT_phase(inst, prev[0], prev[1], sT)
                    prev = (db, m_sb)
                sT_phase(inst, prev[0], prev[1], sT)
                return sT

            def stats_exp(inst, sT):
                """InstanceNorm stats -> exp in place on sT."""
                stats = stats2[:, inst, :]
                red = smallp.tile([128, 2], dt.float32, tag="red", bufs=2)
                nc.vector.tensor_reduce(
                    out=red[:, 0:1], in_=ssum[:, inst, :],
                    axis=mybir.AxisListType.X, op=ALU.add,
                )
                nc.vector.tensor_reduce(
                    out=red[:, 1:2], in_=ssq[:, inst, :],
                    axis=mybir.AxisListType.X, op=ALU.add,
                )
                red_r = smallp.tile([128, 2], dt.float32r, tag="red_r", bufs=2)
                nc.vector.tensor_copy(out=red_r[:], in_=red[:])
                ptr = psp.tile([128, 512], dt.float32, tag="pp")
                nc.tensor.matmul(
                    ptr[:, 0:2], ones_r[:], red_r[:], start=True, stop=True
                )
                nc.scalar.activation(
                    stats[:, 0:2], ptr[:, 0:2], AF.Copy, bias=0.0,
                    scale=1.0 / PLANE,
                )
                mu = stats[:, 0:1]
                ex2 = stats[:, 1:2]
                musq = stats[:, 2:3]
                var = stats[:, 3:4]
                std = stats[:, 4:5]
                rstd = stats[:, 5:6]
                nmr = stats[:, 6:7]
                nc.vector.tensor_tensor(out=musq, in0=mu, in1=mu, op=ALU.mult)
                nc.vector.tensor_tensor(out=var, in0=ex2, in1=musq,
                                        op=ALU.subtract)
                nc.vector.tensor_scalar_add(var, var, EPS)
                nc.scalar.activation(std, var, AF.Sqrt, bias=0.0)
                nc.vector.reciprocal(rstd, std)
                nc.vector.tensor_tensor(out=nmr, in0=mu, in1=rstd, op=ALU.mult)
                nc.scalar.mul(nmr, nmr, -1.0)
                for kb in range(NB):
                    nc.scalar.activation(
                        sT[:, kb, :], sT[:, kb, :], AF.Exp,
                        bias=nmr, scale=rstd,
                    )

            def den_pass(inst, sT):
                """softmax denominator per ch -> invden2[:, inst, :]."""
                ps_den = psp.tile([128, 512], dt.float32, tag="pp",
                                  name="ps_den")
                for kb in range(NB):
                    nc.tensor.matmul(
                        ps_den[0:1, :], onescol[:], sT[:, kb, :],
                        start=(kb == 0), stop=(kb == NB - 1),
                    )
                den_sb = smallp.tile([1, CH], BF, tag="den_sb", bufs=2)
                nc.vector.tensor_copy(out=den_sb[:], in_=ps_den[0:1, :])
                ptd = psp.tile([128, 512], dt.float32, tag="pp", name="ptd")
                for cb in range(CB):
                    nc.tensor.matmul(
                        ptd[:, cb:cb + 1],
                        den_sb[0:1, cb * 128:(cb + 1) * 128],
                        onescol[0:1, 0:1],
                        start=True, stop=True,
                    )
                denT = smallp.tile([128, CB], dt.float32, tag="denT", bufs=2)
                nc.vector.tensor_copy(out=denT[:], in_=ptd[:, 0:CB])
                nc.vector.reciprocal(invden2[:, inst, :], denT[:])

            etp_tiles = []

            def load_etp():
                for k in range(4):
                    etp = streamp.tile([128, N], BF, tag="etp", bufs=4)
                    nc.sync.dma_start_transpose(etp[:], emb_bf[k])
                    etp_tiles.append(etp)

            def wva_phase(inst, db, sT):
                pwa = psp.tile([128, 512], dt.float32, tag="pp", name="pwa")
                for chb in range(CB):
                    for j in range(CB):
                        nc.tensor.matmul(
                            pwa[:, chb * C:(chb + 1) * C],
                            sT[:, db * CB + j, chb * 128:(chb + 1) * 128],
                            WvT_b[:, j, :],
                            start=(j == 0),
                            stop=(j == CB - 1),
                        )
                wva = streamp.tile([128, CB, C], BF, tag="wva")
                for chb in range(CB):
                    nc.vector.tensor_scalar_mul(
                        wva[:, chb, :],
                        pwa[:, chb * C:(chb + 1) * C],
                        invden2[:, inst, chb:chb + 1],
                    )
                return wva

            def ctx_out(inst, sT):
                """outT = sum_db (wvo_db)^T @ emb_db^T."""
                pwo = psp.tile([128, 512], dt.float32, tag="pp", name="pwo")

                def wvo_phase(db, wva):
                    for chb in range(CB):
                        nc.tensor.matmul(
                            pwo[0:C, db * C:(db + 1) * C],
                            wva[:, chb, :],
                            Wout_b[:, chb, :],
                            start=(chb == 0),
                            stop=(chb == CB - 1),
                        )

                prev = None
                for db in range(B):
                    wva = wva_phase(inst, db, sT)
                    if prev is not None:
                        wvo_phase(prev[0], prev[1])
                    prev = (db, wva)
                wvo_phase(prev[0], prev[1])
                # pair-stack wvo: [128(2 batches' c'), 4(pair), 64]
                wvo2 = streamp.tile([128, CB, C], BF, tag="wvo2")
                for db in range(B):
                    nc.vector.tensor_copy(
                        out=wvo2[(db % 2) * C:(db % 2 + 1) * C, db // 2, :],
                        in_=pwo[0:C, db * C:(db + 1) * C],
                    )
                # outT[c, n] = sum_pairs wvo_pair^T @ embT_pair
                for g in range(8):
                    pout = psp.tile([128, 512], dt.float32, tag="pp",
                                    name="pout")
                    for k in range(4):
                        nc.tensor.matmul(
                            pout[0:C, :],
                            wvo2[:, k, :],
                            etp_tiles[k][:, g * 512:(g + 1) * 512],
                            start=(k == 0),
                            stop=(k == 3),
                        )
                    ot = streamp.tile([C, 512], dt.float32, tag="ot")
                    nc.vector.tensor_copy(out=ot[:], in_=pout[0:C, :])
                    nc.sync.dma_start(
                        out_d[inst, :, g * 512:(g + 1) * 512], ot[:]
                    )

            # ================= schedule =================
            to_bf(0)
            build_q(0)
            for k in range(1, 4):
                to_bf(k)
            sT0 = s_pass(0, [None] * 4)
            build_q(1)
            stats_exp(0, sT0)
            sT1 = s_pass(1, [None] * 4)
            load_etp()
            den_pass(0, sT0)
            stats_exp(1, sT1)
            ctx_out(0, sT0)
            den_pass(1, sT1)
            ctx_out(1, sT1)

    nc.compile()
    return nc


def _get_nc():
    global _nc
    if _nc is None:
        _nc = _build()
    return _nc


def kernel(emb, Wq, Wk, Wv, Wout):
    emb = np.ascontiguousarray(emb, dtype=np.float32)
    Wq = np.ascontiguousarray(Wq, dtype=np.float32)
    Wk = np.ascontiguousarray(Wk, dtype=np.float32)
    Wv = np.ascontiguousarray(Wv, dtype=np.float32)
    Wout = np.ascontiguousarray(Wout, dtype=np.float32)
    emb_l, emb_u = emb[:B], emb[B:]
    ident = np.eye(128, dtype=np.float32)
    ones = np.ones((128, 128), dtype=np.float32)

    in_maps = []
    for core in range(8):
        if core < 4:
            qb, kvb = emb_l[2 * core:2 * core + 2], emb_u
        else:
            j = core - 4
            qb, kvb = emb_u[2 * j:2 * j + 2], emb_l
        in_maps.append({
            "embq": np.ascontiguousarray(qb), "embkv": np.ascontiguousarray(kvb),
            "Wq": Wq, "Wk": Wk, "Wv": Wv, "Wout": Wout, "ident": ident,
            "ones": ones,
        })

    res = run_bass_kernel_spmd(_get_nc(), in_maps, list(range(8))).results

    out = np.empty((2 * B, N, C), np.float32)
    for core in range(8):
        o = res[core]["out"].transpose(0, 2, 1)  # [2, C, N] -> [2, N, C]
        if core < 4:
            out[2 * core:2 * core + 2] = o
        else:
            j = core - 4
            out[B + 2 * j:B + 2 * j + 2] = o
    return out
